# revision 1
# baseline (speedup 1.0000x reference)
"""GraphSAGE-style 3-layer GNN (mean aggregation) on 8 Trainium2 NeuronCores.

Strategy (dst-sharded graph parallelism):
- Nodes (and their incoming edges) are sharded across 8 cores: core d owns
  dst nodes [d*6250, (d+1)*6250).
- Host preprocessing sorts each core's edges by dst node-tile (128 dst nodes
  per tile) and packs them into fixed 128-edge chunks on a uniform
  [49 tiles x M chunks] grid (padded with weight-0 edges).
- Aggregation on device: for each chunk, a one-hot matrix
  OH[e, n] = (dstloc[e] == n) * (1/deg) is built on the DVE from an iota tile,
  and  meanT[c, n] += G[e, c].T @ OH[e, n]  accumulates in PSUM on the PE.
- Layer-1 edge source features are pre-gathered on the host (edge-parallel
  input sharding) and streamed sequentially; layers 2/3 gather their source
  features from a replicated DRAM table via indirect DMA (128 rows/call).
- The replicated table is refreshed between layers with an AllGather
  (halo exchange) over the 8 cores.
- Dense transforms run in transposed layout: hT = Wl.T @ meanT + Wr.T @ xT,
  bias add + ReLU on the DVE, then PE-transposes produce row-major h for the
  next layer's gather table.
Compute dtype: bf16 inputs with fp32 PSUM accumulation.
"""
import numpy as np
import ml_dtypes

import concourse.bass as bass
import concourse.bacc as bacc
import concourse.mybir as mybir
import concourse.tile as tile
from concourse.bass_utils import run_bass_kernel_spmd

N = 50000
E = 800000
C1 = 128
HID = 256
OUT = 15
NCORES = 8
NP = N // NCORES          # 6250 own nodes per core
P = 128
TILES = (NP + P - 1) // P  # 49 node tiles per core
NPAD = TILES * P           # 6272
KB = 6                     # L1 staged-gather chunks per DMA

bf16 = mybir.dt.bfloat16
f32 = mybir.dt.float32
_bf = ml_dtypes.bfloat16
ABLATE = set()  # {"dense","rows","aggmm","gather","coll"} for timing experiments
# defaults: host-precomputed one-hot matrices streamed from DRAM instead of
# DVE builds, and int16 dma_gather (lo/hi split tables, <=768-descriptor calls)
# instead of per-chunk indirect DMAs for the layer-2/3 feature gathers
OPT = {"hostoh", "antgather", "deep"}  # also: {"batchgather","sharedhf"}
KB2 = 6         # chunks per batched indirect gather (768 descriptors)
SPLIT = 32768   # lo/hi table split for int16 dma_gather indices


def _pack16(a):
    """Pack [TILES, L] int row arrays into the dma_gather int16 index layout:
    position i lives at idx16[i % 16, i // 16], replicated to 128 partitions."""
    TILESn, L = a.shape
    S = L // 16
    out = np.zeros((TILESn, 16, S), np.int16)
    i = np.arange(L)
    out[:, i % 16, i // 16] = a.astype(np.int16)
    rep = np.tile(out, (1, 8, 1))                      # [TILES, 128, S]
    return np.ascontiguousarray(rep.transpose(1, 0, 2).reshape(128, TILESn * S))


def _preprocess(x, edge_index):
    """Sort/pad edges into the uniform [NCORES, TILES, M, 128] chunk grid and
    build all per-core staged arrays. Within each tile, chunks are grouped
    lo-src (src < SPLIT) first, then hi-src, so layers 2/3 can gather each
    group with one int16 dma_gather per tile."""
    src = np.ascontiguousarray(edge_index[0]).astype(np.int64)
    dst = np.ascontiguousarray(edge_index[1]).astype(np.int64)
    deg = np.bincount(dst, minlength=N)
    wnode = (1.0 / np.maximum(deg, 1)).astype(np.float32)

    core = dst // NP
    tilei = (dst - core * NP) // P
    gkey = core * TILES + tilei
    okey = gkey * 2 + (src >= SPLIT)
    order = np.argsort(okey, kind="stable")
    so = okey[order]
    ssrc = src[order]
    sdst = dst[order]
    bounds = np.searchsorted(so, np.arange(NCORES * TILES * 2))
    counts2 = np.diff(np.append(bounds, E))
    MLO = int((counts2[0::2].max() + P - 1) // P)
    MHI = int((counts2[1::2].max() + P - 1) // P)
    M = MLO + MHI
    NCH = TILES * M

    # slot grids: lo edges at chunks [0, MLO), hi at [MLO, M) of each tile
    rank = np.arange(E) - bounds[so]
    gk = so // 2
    ishi = so % 2
    slot = gk * (M * P) + ishi * (MLO * P) + rank
    src_grid = np.zeros(NCORES * NCH * P, np.int64)
    # hi-chunk padding rows must stay in the hi table: default row = SPLIT
    src_grid.reshape(NCORES, TILES, M, P)[:, :, MLO:, :] = SPLIT
    dstloc_grid = np.full(NCORES * NCH * P, P - 1, np.int64)
    w_grid = np.zeros(NCORES * NCH * P, np.float32)
    src_grid[slot] = ssrc
    dstloc_grid[slot] = sdst - (sdst // NP) * NP - ((sdst - (sdst // NP) * NP) // P) * P
    w_grid[slot] = wnode[sdst]

    src_grid = src_grid.reshape(NCORES, NCH, P)
    dstloc_grid = dstloc_grid.reshape(NCORES, NCH, P)
    w_grid = w_grid.reshape(NCORES, NCH, P)

    x_bf = x.astype(_bf)
    per_core = []
    for d in range(NCORES):
        idx32 = np.ascontiguousarray(src_grid[d].T).astype(np.int32)   # [128, NCH]
        dstw = np.empty((P, 2 * NCH), np.float32)
        dstw[:, 0::2] = dstloc_grid[d].T
        dstw[:, 1::2] = w_grid[d].T
        dstw = dstw.astype(_bf)
        # L1 pre-gathered edge features [128, NCH, C1]
        xg1 = np.ascontiguousarray(x_bf[src_grid[d]].transpose(1, 0, 2))
        # own transposed features [128, NPAD]
        xT = np.zeros((C1, NPAD), np.float32)
        xT[:, :NP] = x[d * NP:(d + 1) * NP].T
        # host-built one-hot scatter grid: oh[lane, chunk*P + dst]
        ohg = np.zeros((NCH, P, P), np.float32)
        ch = np.repeat(np.arange(NCH), P)
        ln = np.tile(np.arange(P), NCH)
        ohg[ch, ln, dstloc_grid[d].ravel()] = w_grid[d].ravel()
        ohg = np.ascontiguousarray(ohg.transpose(1, 0, 2).reshape(P, NCH * P)).astype(_bf)
        # int16 dma_gather index planes
        s3 = src_grid[d].reshape(TILES, M, P)
        idx16lo = _pack16(s3[:, :MLO, :].reshape(TILES, MLO * P))
        idx16hi = _pack16(s3[:, MLO:, :].reshape(TILES, MHI * P) - SPLIT)
        per_core.append(dict(idx32=idx32, dstw=dstw, xg1=xg1, oh=ohg,
                             idx16lo=idx16lo, idx16hi=idx16hi,
                             xT=np.ascontiguousarray(xT).astype(_bf)))
    return per_core, M, MLO, MHI


def _build(nc: bass.Bass, M: int, MLO: int = 0, MHI: int = 0):
    NCH = TILES * M
    # ---- I/O ----
    idx32_d = nc.dram_tensor("idx32", [P, NCH], mybir.dt.int32, kind="ExternalInput")
    idxlo_d = nc.dram_tensor("idx16lo", [P, TILES * MLO * 8], mybir.dt.int16,
                             kind="ExternalInput")
    idxhi_d = nc.dram_tensor("idx16hi", [P, TILES * MHI * 8], mybir.dt.int16,
                             kind="ExternalInput")
    dstw_d = nc.dram_tensor("dstw", [P, 2 * NCH], bf16, kind="ExternalInput")
    oh_d = nc.dram_tensor("oh", [P, NCH * P], bf16, kind="ExternalInput")
    xg1_d = nc.dram_tensor("xg1", [P, NCH, C1], bf16, kind="ExternalInput")
    xT_d = nc.dram_tensor("xT", [P, NPAD], bf16, kind="ExternalInput")
    iota_d = nc.dram_tensor("iota", [P, P], bf16, kind="ExternalInput")
    wl_d = {}
    wr_d = {}
    for l, cin in ((1, C1), (2, HID), (3, HID)):
        wl_d[l] = nc.dram_tensor(f"Wl{l}", [cin, HID], bf16, kind="ExternalInput")
        wr_d[l] = nc.dram_tensor(f"Wr{l}", [cin, HID], bf16, kind="ExternalInput")
    bl_d = nc.dram_tensor("bl", [P, 6], f32, kind="ExternalInput")       # [:, 2(l-1)+j]
    wo_d = nc.dram_tensor("Wo", [HID, OUT], bf16, kind="ExternalInput")
    bo_d = nc.dram_tensor("bo", [1, OUT], f32, kind="ExternalInput")
    out_d = nc.dram_tensor("out", [NP, OUT], f32, kind="ExternalOutput")

    KH = HID // P  # 2 halves of the hidden dim
    nblocks = [(b, min(512, NPAD - b)) for b in range(0, NPAD, 512)]

    with tile.TileContext(nc) as tc:
        with (
            tc.tile_pool(name="const", bufs=1) as cst,
            tc.tile_pool(name="feat", bufs=1) as featp,
            tc.tile_pool(name="g", bufs=24) as gp,
            tc.tile_pool(name="oh", bufs=6 if "deep" in OPT else 4) as ohp,
            tc.tile_pool(name="stage", bufs=4) as stp,
            tc.tile_pool(name="pmean", bufs=2, space="PSUM") as pmean,
            tc.tile_pool(name="pdense", bufs=2, space="PSUM") as pdense,
            tc.tile_pool(name="ptr", bufs=1, space="PSUM") as ptr,
            tc.tile_pool(name="dram", bufs=1, space="DRAM") as dr,
        ):
            # ---- constants (funneled through DVE so consumers carry few waits)
            _cid = [0]
            def load_const(shape, dt, src_ap):
                _cid[0] += 1
                ld = cst.tile(shape, dt, tag=f"cld{_cid[0]}", name=f"cld{_cid[0]}")
                nc.sync.dma_start(ld[:], src_ap)
                t = cst.tile(shape, dt, tag=f"cst{_cid[0]}", name=f"cst{_cid[0]}")
                nc.vector.tensor_copy(t[:], ld[:])
                return t

            if "hostoh" not in OPT:
                iota_sb = load_const([P, P], bf16, iota_d[:])
                dstw_sb = load_const([P, 2 * NCH], bf16, dstw_d[:])
            if "antgather" in OPT:
                idxlo_sb = cst.tile([P, TILES * MLO * 8], mybir.dt.int16)
                nc.sync.dma_start(idxlo_sb[:], idxlo_d[:])
                idxhi_sb = cst.tile([P, TILES * MHI * 8], mybir.dt.int16)
                nc.sync.dma_start(idxhi_sb[:], idxhi_d[:])
            else:
                idx_sb = load_const([P, NCH], mybir.dt.int32, idx32_d[:])
            bl_sb = load_const([P, 6], f32, bl_d[:])
            wo_sb = [load_const([P, OUT], bf16, wo_d[h * P:(h + 1) * P, :])
                     for h in range(HID // P)]
            # bo broadcast to 128 partitions via DMA step-0
            bo_ld = cst.tile([P, OUT], f32)
            nc.sync.dma_start(bo_ld[:], bo_d[0:1, :].to_broadcast([P, OUT]))
            bo_sb = cst.tile([P, OUT], f32)
            nc.vector.tensor_copy(bo_sb[:], bo_ld[:])
            wl_sb = {}
            wr_sb = {}
            for l, cin in ((1, C1), (2, HID), (3, HID)):
                wl_sb[l] = [load_const([P, HID], bf16, wl_d[l][h * P:(h + 1) * P, :])
                            for h in range(cin // P)]
                wr_sb[l] = [load_const([P, HID], bf16, wr_d[l][h * P:(h + 1) * P, :])
                            for h in range(cin // P)]
            identity = cst.tile([P, P], bf16)
            from concourse.masks import make_identity
            make_identity(nc, identity[:])

            # ---- feature double buffers (transposed layout, [128, NPAD] per half)
            xT_sb = [featp.tile([P, NPAD], bf16, tag=f"ft0_{h}", name=f"xT_sb{h}") for h in range(KH)]
            hT_sb = [featp.tile([P, NPAD], bf16, tag=f"ft1_{h}", name=f"hT_sb{h}") for h in range(KH)]
            meanT_sb = [featp.tile([P, NPAD], bf16, tag=f"mt_{h}", name=f"meanT_sb{h}") for h in range(KH)]
            nc.sync.dma_start(xT_sb[0][:], xT_d[:])

            # DRAM halo buffers (ping-pong: Shared tensors allow 1 writer each)
            h_own = dr.tile([NP, HID], bf16)
            if "sharedhf" in OPT:
                h_fullA = dr.tile([N, HID], bf16, addr_space="Shared", name="h_fullA")
                h_fullB = dr.tile([N, HID], bf16, addr_space="Shared", name="h_fullB")
            else:
                h_fullA = h_fullB = dr.tile([N, HID], bf16, name="h_full")
            h_full_for = {2: h_fullA, 3: h_fullB}

            def build_oh(t):
                """One-hot block for node-tile t: [128, M*128] bf16."""
                oh = ohp.tile([P, M * P], bf16, tag="oh")
                if "hostoh" in OPT:
                    # host-precomputed scatter matrices streamed from DRAM
                    nc.sync.dma_start(oh[:], oh_d[:, t * M * P:(t + 1) * M * P])
                    return oh
                dloc = dstw_sb[:, 2 * t * M:2 * (t + 1) * M:2]          # [128, M]
                wcol = dstw_sb[:, 2 * t * M + 1:2 * (t + 1) * M:2]      # [128, M]
                nc.vector.tensor_tensor(
                    out=oh[:].rearrange("p (m n) -> p m n", m=M),
                    in0=dloc[:, :, None].to_broadcast([P, M, P]),
                    in1=iota_sb[:, None, :].to_broadcast([P, M, P]),
                    op=mybir.AluOpType.is_equal)
                nc.vector.tensor_tensor(
                    out=oh[:].rearrange("p (m n) -> p m n", m=M),
                    in0=oh[:].rearrange("p (m n) -> p m n", m=M),
                    in1=wcol[:, :, None].to_broadcast([P, M, P]),
                    op=mybir.AluOpType.mult)
                return oh

            def aggregate(layer, cin):
                """meanT_sb <- segment-mean of gathered source features."""
                khalves = cin // P
                h_full = h_full_for.get(layer)
                for t in range(TILES):
                    oh = build_oh(t)
                    pm = [pmean.tile([P, P], f32, tag=f"pm{h}", space="PSUM",
                                     name=f"pm_{t}_{h}") for h in range(khalves)]
                    if layer == 1:
                        for mb_ in range(0, M, KB):
                            nb = min(KB, M - mb_)
                            g = gp.tile([P, KB * C1], bf16, tag="g1")
                            nc.scalar.dma_start(
                                g[:, :nb * C1],
                                xg1_d[:, t * M + mb_:t * M + mb_ + nb, :])
                            for j in range(nb):
                                m = mb_ + j
                                if "aggmm" in ABLATE and m > 0:
                                    continue
                                nc.tensor.matmul(
                                    pm[0][:], lhsT=g[:, j * C1:(j + 1) * C1],
                                    rhs=oh[:, m * P:(m + 1) * P],
                                    start=(m == 0),
                                    stop=(m == M - 1 or "aggmm" in ABLATE))
                    elif "antgather" in OPT:
                        # dma_gather calls capped at KB2 chunks (<=768
                        # descriptors) so one call never exceeds the 1024-desc
                        # SWDGE carveout ring
                        Slo, Shi = MLO * 8, MHI * 8
                        chunk_src = []   # (g_tile, local_m) for m = 0..M-1
                        for view, idxsb, scol, Mx in (
                                (h_full[0:SPLIT, :], idxlo_sb, t * Slo, MLO),
                                (h_full[SPLIT:N, :], idxhi_sb, t * Shi, MHI)):
                            for base_ in range(0, Mx, KB2):
                                nb = min(KB2, Mx - base_)
                                g = gp.tile([P, KB2, HID], bf16, tag="gant",
                                            bufs=14 if "deep" in OPT else 8)
                                nc.gpsimd.dma_gather(
                                    out_ap=g[:, :nb, :], in_ap=view,
                                    idxs_ap=idxsb[:, scol + base_ * 8:
                                                  scol + (base_ + nb) * 8],
                                    num_idxs=nb * P, num_idxs_reg=nb * P,
                                    elem_size=HID)
                                chunk_src += [(g, j) for j in range(nb)]
                        for m in range(M):
                            if "aggmm" in ABLATE and m > 0:
                                continue
                            gsrc, mm = chunk_src[m]
                            for h in range(khalves):
                                nc.tensor.matmul(
                                    pm[h][:],
                                    lhsT=gsrc[:, mm, h * P:(h + 1) * P],
                                    rhs=oh[:, m * P:(m + 1) * P],
                                    start=(m == 0),
                                    stop=(m == M - 1 or "aggmm" in ABLATE))
                    elif "batchgather" in OPT:
                        for mb_ in range(0, M, KB2):
                            nb = min(KB2, M - mb_)
                            k = t * M + mb_
                            g = gp.tile([P, KB2 * HID], bf16, tag="gb", bufs=6)
                            nc.gpsimd.indirect_dma_start(
                                out=g[:, :nb * HID], out_offset=None,
                                in_=h_full[:],
                                in_offset=bass.IndirectOffsetOnAxis(
                                    ap=idx_sb[:, k:k + nb], axis=0))
                            for j in range(nb):
                                m = mb_ + j
                                if "aggmm" in ABLATE and m > 0:
                                    continue
                                for h in range(khalves):
                                    nc.tensor.matmul(
                                        pm[h][:],
                                        lhsT=g[:, j * HID + h * P:j * HID + (h + 1) * P],
                                        rhs=oh[:, m * P:(m + 1) * P],
                                        start=(m == 0),
                                        stop=(m == M - 1 or "aggmm" in ABLATE))
                    else:
                        for m in range(M):
                            k = t * M + m
                            g = gp.tile([P, HID], bf16, tag="g2")
                            if "gather" in ABLATE:
                                # same-volume sequential DMA instead of
                                # random indirect gather
                                nc.scalar.dma_start(
                                    g[:], h_full[(k % 390) * P:(k % 390) * P + P, :])
                            else:
                                nc.gpsimd.indirect_dma_start(
                                    out=g[:], out_offset=None, in_=h_full[:],
                                    in_offset=bass.IndirectOffsetOnAxis(
                                        ap=idx_sb[:, k:k + 1], axis=0))
                            if "aggmm" in ABLATE and m > 0:
                                continue
                            for h in range(khalves):
                                nc.tensor.matmul(
                                    pm[h][:], lhsT=g[:, h * P:(h + 1) * P],
                                    rhs=oh[:, m * P:(m + 1) * P],
                                    start=(m == 0),
                                    stop=(m == M - 1 or "aggmm" in ABLATE))
                    for h in range(khalves):
                        nc.vector.tensor_copy(
                            meanT_sb[h][:, t * P:(t + 1) * P], pm[h][:])

            def dense(layer, cin, src_feat, dst_feat):
                """dst_feat[j] = relu(Wl.T @ meanT + Wr.T @ src_feat + bl)."""
                khalves = cin // P
                for j in range(KH):
                    for b0, blen in nblocks:
                        pd = pdense.tile([P, 512], f32, tag="pd", space="PSUM")
                        nmm = 2 * khalves if "dense" not in ABLATE else 1
                        i = 0
                        for h in range(khalves):
                            nc.tensor.matmul(
                                pd[:, :blen],
                                lhsT=wl_sb[layer][h][:, j * P:(j + 1) * P],
                                rhs=meanT_sb[h][:, b0:b0 + blen],
                                start=(i == 0), stop=(i == nmm - 1)); i += 1
                            if "dense" in ABLATE:
                                break
                            nc.tensor.matmul(
                                pd[:, :blen],
                                lhsT=wr_sb[layer][h][:, j * P:(j + 1) * P],
                                rhs=src_feat[h][:, b0:b0 + blen],
                                start=(i == 0), stop=(i == nmm - 1)); i += 1
                        nc.vector.tensor_scalar(
                            out=dst_feat[j][:, b0:b0 + blen], in0=pd[:, :blen],
                            scalar1=bl_sb[:, 2 * (layer - 1) + j:2 * (layer - 1) + j + 1],
                            scalar2=0.0,
                            op0=mybir.AluOpType.add, op1=mybir.AluOpType.max)

            def write_rows(feat, h_full):
                """Transpose hT -> row-major h_own, then AllGather into h_full."""
                for t in range(TILES):
                    rows = stp.tile([P, HID], bf16, tag="rows")
                    for j in range(KH):
                        if "rows" in ABLATE:  # skip PE transpose, straight copy
                            nc.vector.tensor_copy(
                                rows[:, j * P:(j + 1) * P],
                                feat[j][:, t * P:(t + 1) * P])
                            continue
                        pt = ptr.tile([P, P], bf16, tag="pt", space="PSUM")
                        nc.tensor.transpose(
                            pt[:], feat[j][:, t * P:(t + 1) * P], identity[:])
                        nc.vector.tensor_copy(rows[:, j * P:(j + 1) * P], pt[:])
                    nrow = min(P, NP - t * P)
                    nc.scalar.dma_start(h_own[t * P:t * P + nrow, :], rows[:nrow, :])
                if h_full is not None and "coll" not in ABLATE:
                    nc.gpsimd.collective_compute(
                        "AllGather", mybir.AluOpType.bypass,
                        replica_groups=[list(range(NCORES))],
                        ins=[h_own[:]], outs=[h_full[:]])

            # ---- layer 1
            aggregate(1, C1)
            dense(1, C1, xT_sb, hT_sb)
            write_rows(hT_sb, h_full_for[2])
            # ---- layer 2
            aggregate(2, HID)
            dense(2, HID, hT_sb, xT_sb)   # ping-pong: xT_sb now holds h2T
            write_rows(xT_sb, h_full_for[3])
            # ---- layer 3
            aggregate(3, HID)
            dense(3, HID, xT_sb, hT_sb)   # hT_sb now holds h3T
            # ---- output layer: out[n, :] = h3.T @ Wo + bo
            for t in range(TILES):
                po = ptr.tile([P, OUT], f32, tag="po", space="PSUM")
                for h in range(KH):
                    nc.tensor.matmul(
                        po[:], lhsT=hT_sb[h][:, t * P:(t + 1) * P],
                        rhs=wo_sb[h][:],
                        start=(h == 0), stop=(h == KH - 1))
                orow = stp.tile([P, OUT], f32, tag="orow")
                nc.vector.tensor_tensor(out=orow[:], in0=po[:], in1=bo_sb[:],
                                        op=mybir.AluOpType.add)
                nrow = min(P, NP - t * P)
                nc.sync.dma_start(out_d[t * P:t * P + nrow, :], orow[:nrow, :])
    return nc


_PROGRAM_CACHE = {}


def _get_program(meta):
    key = (meta, frozenset(OPT), frozenset(ABLATE))
    if key not in _PROGRAM_CACHE:
        nc = bacc.Bacc("TRN2", target_bir_lowering=False, debug=False,
                       num_devices=NCORES)
        _build(nc, *meta)
        nc.compile()
        _PROGRAM_CACHE[key] = nc
    return _PROGRAM_CACHE[key]


def make_in_maps(inputs):
    x = np.asarray(inputs["x"], np.float32)
    per_core, M, MLO, MHI = _preprocess(x, np.asarray(inputs["edge_index"]))
    iota = np.tile(np.arange(P, dtype=np.float32)[None, :], (P, 1)).astype(_bf)
    bl = np.zeros((P, 6), np.float32)
    for l in (1, 2, 3):
        b = np.asarray(inputs[f"bl{l}"], np.float32)
        bl[:, 2 * (l - 1)] = b[:P]
        bl[:, 2 * (l - 1) + 1] = b[P:]
    common = {"iota": iota, "bl": bl,
              "Wo": np.asarray(inputs["Wo"]).astype(_bf),
              "bo": np.asarray(inputs["bo"], np.float32).reshape(1, OUT)}
    for l in (1, 2, 3):
        common[f"Wl{l}"] = np.asarray(inputs[f"Wl{l}"]).astype(_bf)
        common[f"Wr{l}"] = np.asarray(inputs[f"Wr{l}"]).astype(_bf)
    in_maps = []
    for d in range(NCORES):
        pc = per_core[d]
        in_maps.append({**common, "idx32": pc["idx32"], "dstw": pc["dstw"],
                        "xg1": pc["xg1"], "xT": pc["xT"], "oh": pc["oh"],
                        "idx16lo": pc["idx16lo"], "idx16hi": pc["idx16hi"]})
    return in_maps, (M, MLO, MHI)


def kernel(**inputs) -> np.ndarray:
    in_maps, meta = make_in_maps(inputs)
    nc = _get_program(meta)
    res = run_bass_kernel_spmd(nc, in_maps, core_ids=list(range(NCORES)))
    out = np.concatenate(
        [np.asarray(res.results[d]["out"], np.float32) for d in range(NCORES)], axis=0)
    return out



# revision 7
# speedup vs baseline: 2.3474x; 2.3474x over previous
"""GraphSAGE-style 3-layer GNN (mean aggregation) on 8 Trainium2 NeuronCores.

Strategy (dst-sharded graph parallelism):
- Nodes (and their incoming edges) are sharded across 8 cores: core d owns
  dst nodes [d*6250, (d+1)*6250).
- Host preprocessing sorts each core's edges by dst node-tile (128 dst nodes
  per tile) and packs them into fixed 128-edge chunks on a uniform
  [49 tiles x M chunks] grid (padded with weight-0 edges).
- Aggregation on device: for each chunk, a one-hot matrix
  OH[e, n] = (dstloc[e] == n) * (1/deg) is built on the DVE from an iota tile,
  and  meanT[c, n] += G[e, c].T @ OH[e, n]  accumulates in PSUM on the PE.
- Layer-1 edge source features are pre-gathered on the host (edge-parallel
  input sharding) and streamed sequentially; layers 2/3 gather their source
  features from a replicated DRAM table via indirect DMA (128 rows/call).
- The replicated table is refreshed between layers with an AllGather
  (halo exchange) over the 8 cores.
- Dense transforms run in transposed layout: hT = Wl.T @ meanT + Wr.T @ xT,
  bias add + ReLU on the DVE, then PE-transposes produce row-major h for the
  next layer's gather table.
Compute dtype: bf16 inputs with fp32 PSUM accumulation.
"""
import numpy as np
import ml_dtypes

import concourse.bass as bass
import concourse.bacc as bacc
import concourse.mybir as mybir
import concourse.tile as tile
from concourse.bass_utils import run_bass_kernel_spmd

N = 50000
E = 800000
C1 = 128
HID = 256
OUT = 15
NCORES = 8
NP = N // NCORES          # 6250 own nodes per core
P = 128
TILES = (NP + P - 1) // P  # 49 node tiles per core
NPAD = TILES * P           # 6272
KB = 6                     # L1 staged-gather chunks per DMA

bf16 = mybir.dt.bfloat16
f32 = mybir.dt.float32
_bf = ml_dtypes.bfloat16
ABLATE = set()  # {"dense","rows","aggmm","gather","coll"} for timing experiments
# defaults: host-precomputed one-hot matrices streamed from DRAM instead of
# DVE builds, and int16 dma_gather (lo/hi split tables, <=768-descriptor calls)
# instead of per-chunk indirect DMAs for the layer-2/3 feature gathers
OPT = {"hostoh", "antgather", "deep", "sharedhf"}  # also: {"batchgather"}
NSWQ = 4        # SWDGE queues; dma_gather calls round-robin across them
KB2 = 6         # chunks per batched indirect gather (768 descriptors)
SPLIT = 32768   # lo/hi table split for int16 dma_gather indices


def _pack16(a):
    """Pack [TILES, L] int row arrays into the dma_gather int16 index layout:
    position i lives at idx16[i % 16, i // 16], replicated to 128 partitions."""
    TILESn, L = a.shape
    S = L // 16
    out = np.zeros((TILESn, 16, S), np.int16)
    i = np.arange(L)
    out[:, i % 16, i // 16] = a.astype(np.int16)
    rep = np.tile(out, (1, 8, 1))                      # [TILES, 128, S]
    return np.ascontiguousarray(rep.transpose(1, 0, 2).reshape(128, TILESn * S))


def _preprocess(x, edge_index):
    """Sort/pad edges into the uniform [NCORES, TILES, M, 128] chunk grid and
    build all per-core staged arrays. Within each tile, chunks are grouped
    lo-src (src < SPLIT) first, then hi-src, so layers 2/3 can gather each
    group with one int16 dma_gather per tile."""
    src = np.ascontiguousarray(edge_index[0]).astype(np.int64)
    dst = np.ascontiguousarray(edge_index[1]).astype(np.int64)
    deg = np.bincount(dst, minlength=N)
    wnode = (1.0 / np.maximum(deg, 1)).astype(np.float32)

    core = dst // NP
    tilei = (dst - core * NP) // P
    gkey = core * TILES + tilei
    okey = gkey * 2 + (src >= SPLIT)
    order = np.argsort(okey, kind="stable")
    so = okey[order]
    ssrc = src[order]
    sdst = dst[order]
    bounds = np.searchsorted(so, np.arange(NCORES * TILES * 2))
    counts2 = np.diff(np.append(bounds, E))
    MLO = int((counts2[0::2].max() + P - 1) // P)
    MHI = int((counts2[1::2].max() + P - 1) // P)
    M = MLO + MHI
    NCH = TILES * M

    # slot grids: lo edges at chunks [0, MLO), hi at [MLO, M) of each tile
    rank = np.arange(E) - bounds[so]
    gk = so // 2
    ishi = so % 2
    slot = gk * (M * P) + ishi * (MLO * P) + rank
    src_grid = np.zeros(NCORES * NCH * P, np.int64)
    # hi-chunk padding rows must stay in the hi table: default row = SPLIT
    src_grid.reshape(NCORES, TILES, M, P)[:, :, MLO:, :] = SPLIT
    dstloc_grid = np.full(NCORES * NCH * P, P - 1, np.int64)
    w_grid = np.zeros(NCORES * NCH * P, np.float32)
    src_grid[slot] = ssrc
    dstloc_grid[slot] = sdst - (sdst // NP) * NP - ((sdst - (sdst // NP) * NP) // P) * P
    w_grid[slot] = wnode[sdst]

    src_grid = src_grid.reshape(NCORES, NCH, P)
    dstloc_grid = dstloc_grid.reshape(NCORES, NCH, P)
    w_grid = w_grid.reshape(NCORES, NCH, P)

    x_bf = x.astype(_bf)
    per_core = []
    for d in range(NCORES):
        idx32 = np.ascontiguousarray(src_grid[d].T).astype(np.int32)   # [128, NCH]
        dstw = np.empty((P, 2 * NCH), np.float32)
        dstw[:, 0::2] = dstloc_grid[d].T
        dstw[:, 1::2] = w_grid[d].T
        dstw = dstw.astype(_bf)
        # L1 pre-gathered edge features [128, NCH, C1]
        xg1 = np.ascontiguousarray(x_bf[src_grid[d]].transpose(1, 0, 2))
        # own transposed features [128, NPAD]
        xT = np.zeros((C1, NPAD), np.float32)
        xT[:, :NP] = x[d * NP:(d + 1) * NP].T
        # host-built one-hot scatter grid: oh[lane, chunk*P + dst]
        ohg = np.zeros((NCH, P, P), np.float32)
        ch = np.repeat(np.arange(NCH), P)
        ln = np.tile(np.arange(P), NCH)
        ohg[ch, ln, dstloc_grid[d].ravel()] = w_grid[d].ravel()
        ohg = np.ascontiguousarray(ohg.transpose(1, 0, 2).reshape(P, NCH * P)).astype(_bf)
        # int16 dma_gather index planes
        s3 = src_grid[d].reshape(TILES, M, P)
        idx16lo = _pack16(s3[:, :MLO, :].reshape(TILES, MLO * P))
        idx16hi = _pack16(s3[:, MLO:, :].reshape(TILES, MHI * P) - SPLIT)
        per_core.append(dict(idx32=idx32, dstw=dstw, xg1=xg1, oh=ohg,
                             idx16lo=idx16lo, idx16hi=idx16hi,
                             xT=np.ascontiguousarray(xT).astype(_bf)))
    return per_core, M, MLO, MHI


def _build(nc: bass.Bass, M: int, MLO: int = 0, MHI: int = 0):
    NCH = TILES * M
    # ---- I/O ----
    idx32_d = nc.dram_tensor("idx32", [P, NCH], mybir.dt.int32, kind="ExternalInput")
    idxlo_d = nc.dram_tensor("idx16lo", [P, TILES * MLO * 8], mybir.dt.int16,
                             kind="ExternalInput")
    idxhi_d = nc.dram_tensor("idx16hi", [P, TILES * MHI * 8], mybir.dt.int16,
                             kind="ExternalInput")
    dstw_d = nc.dram_tensor("dstw", [P, 2 * NCH], bf16, kind="ExternalInput")
    oh_d = nc.dram_tensor("oh", [P, NCH * P], bf16, kind="ExternalInput")
    xg1_d = nc.dram_tensor("xg1", [P, NCH, C1], bf16, kind="ExternalInput")
    xT_d = nc.dram_tensor("xT", [P, NPAD], bf16, kind="ExternalInput")
    iota_d = nc.dram_tensor("iota", [P, P], bf16, kind="ExternalInput")
    wl_d = {}
    wr_d = {}
    for l, cin in ((1, C1), (2, HID), (3, HID)):
        wl_d[l] = nc.dram_tensor(f"Wl{l}", [cin, HID], bf16, kind="ExternalInput")
        wr_d[l] = nc.dram_tensor(f"Wr{l}", [cin, HID], bf16, kind="ExternalInput")
    bl_d = nc.dram_tensor("bl", [P, 6], f32, kind="ExternalInput")       # [:, 2(l-1)+j]
    wo_d = nc.dram_tensor("Wo", [HID, OUT], bf16, kind="ExternalInput")
    bo_d = nc.dram_tensor("bo", [1, OUT], f32, kind="ExternalInput")
    out_d = nc.dram_tensor("out", [NP, OUT], f32, kind="ExternalOutput")

    KH = HID // P  # 2 halves of the hidden dim
    nblocks = [(b, min(512, NPAD - b)) for b in range(0, NPAD, 512)]

    with tile.TileContext(nc) as tc:
        with (
            tc.tile_pool(name="const", bufs=1) as cst,
            tc.tile_pool(name="feat", bufs=1) as featp,
            tc.tile_pool(name="g", bufs=24) as gp,
            tc.tile_pool(name="oh", bufs=6 if "deep" in OPT else 4) as ohp,
            tc.tile_pool(name="stage", bufs=4) as stp,
            tc.tile_pool(name="pmean", bufs=2, space="PSUM") as pmean,
            tc.tile_pool(name="pdense", bufs=2, space="PSUM") as pdense,
            tc.tile_pool(name="ptr", bufs=1, space="PSUM") as ptr,
            tc.tile_pool(name="dram", bufs=1, space="DRAM") as dr,
        ):
            # ---- constants (funneled through DVE so consumers carry few waits)
            _cid = [0]
            def load_const(shape, dt, src_ap):
                _cid[0] += 1
                ld = cst.tile(shape, dt, tag=f"cld{_cid[0]}", name=f"cld{_cid[0]}")
                nc.sync.dma_start(ld[:], src_ap)
                t = cst.tile(shape, dt, tag=f"cst{_cid[0]}", name=f"cst{_cid[0]}")
                nc.vector.tensor_copy(t[:], ld[:])
                return t

            if "hostoh" not in OPT:
                iota_sb = load_const([P, P], bf16, iota_d[:])
                dstw_sb = load_const([P, 2 * NCH], bf16, dstw_d[:])
            if "antgather" in OPT:
                idxlo_sb = cst.tile([P, TILES * MLO * 8], mybir.dt.int16)
                nc.sync.dma_start(idxlo_sb[:], idxlo_d[:])
                idxhi_sb = cst.tile([P, TILES * MHI * 8], mybir.dt.int16)
                nc.sync.dma_start(idxhi_sb[:], idxhi_d[:])
            else:
                idx_sb = load_const([P, NCH], mybir.dt.int32, idx32_d[:])
            bl_sb = load_const([P, 6], f32, bl_d[:])
            wo_sb = [load_const([P, OUT], bf16, wo_d[h * P:(h + 1) * P, :])
                     for h in range(HID // P)]
            # bo broadcast to 128 partitions via DMA step-0
            bo_ld = cst.tile([P, OUT], f32)
            nc.sync.dma_start(bo_ld[:], bo_d[0:1, :].to_broadcast([P, OUT]))
            bo_sb = cst.tile([P, OUT], f32)
            nc.vector.tensor_copy(bo_sb[:], bo_ld[:])
            wl_sb = {}
            wr_sb = {}
            for l, cin in ((1, C1), (2, HID), (3, HID)):
                wl_sb[l] = [load_const([P, HID], bf16, wl_d[l][h * P:(h + 1) * P, :])
                            for h in range(cin // P)]
                wr_sb[l] = [load_const([P, HID], bf16, wr_d[l][h * P:(h + 1) * P, :])
                            for h in range(cin // P)]
            identity = cst.tile([P, P], bf16)
            from concourse.masks import make_identity
            make_identity(nc, identity[:])

            # ---- feature double buffers (transposed layout, [128, NPAD] per half)
            xT_sb = [featp.tile([P, NPAD], bf16, tag=f"ft0_{h}", name=f"xT_sb{h}") for h in range(KH)]
            hT_sb = [featp.tile([P, NPAD], bf16, tag=f"ft1_{h}", name=f"hT_sb{h}") for h in range(KH)]
            meanT_sb = [featp.tile([P, NPAD], bf16, tag=f"mt_{h}", name=f"meanT_sb{h}") for h in range(KH)]
            nc.sync.dma_start(xT_sb[0][:], xT_d[:])

            # DRAM halo buffers (ping-pong: Shared tensors allow 1 writer each)
            h_own = dr.tile([NP, HID], bf16)
            if "sharedhf" in OPT:
                h_fullA = dr.tile([N, HID], bf16, addr_space="Shared", name="h_fullA")
                h_fullB = dr.tile([N, HID], bf16, addr_space="Shared", name="h_fullB")
            else:
                h_fullA = h_fullB = dr.tile([N, HID], bf16, name="h_full")
            h_full_for = {2: h_fullA, 3: h_fullB}

            _gq = [0]  # SWDGE queue round-robin state

            def build_oh(t):
                """One-hot block for node-tile t: [128, M*128] bf16."""
                oh = ohp.tile([P, M * P], bf16, tag="oh")
                if "hostoh" in OPT:
                    # host-precomputed scatter matrices streamed from DRAM
                    nc.sync.dma_start(oh[:], oh_d[:, t * M * P:(t + 1) * M * P])
                    return oh
                dloc = dstw_sb[:, 2 * t * M:2 * (t + 1) * M:2]          # [128, M]
                wcol = dstw_sb[:, 2 * t * M + 1:2 * (t + 1) * M:2]      # [128, M]
                nc.vector.tensor_tensor(
                    out=oh[:].rearrange("p (m n) -> p m n", m=M),
                    in0=dloc[:, :, None].to_broadcast([P, M, P]),
                    in1=iota_sb[:, None, :].to_broadcast([P, M, P]),
                    op=mybir.AluOpType.is_equal)
                nc.vector.tensor_tensor(
                    out=oh[:].rearrange("p (m n) -> p m n", m=M),
                    in0=oh[:].rearrange("p (m n) -> p m n", m=M),
                    in1=wcol[:, :, None].to_broadcast([P, M, P]),
                    op=mybir.AluOpType.mult)
                return oh

            def aggregate(layer, cin):
                """meanT_sb <- segment-mean of gathered source features."""
                khalves = cin // P
                h_full = h_full_for.get(layer)
                for t in range(TILES):
                    oh = build_oh(t)
                    pm = [pmean.tile([P, P], f32, tag=f"pm{h}", space="PSUM",
                                     name=f"pm_{t}_{h}") for h in range(khalves)]
                    if layer == 1:
                        for mb_ in range(0, M, KB):
                            nb = min(KB, M - mb_)
                            g = gp.tile([P, KB * C1], bf16, tag="g1")
                            nc.scalar.dma_start(
                                g[:, :nb * C1],
                                xg1_d[:, t * M + mb_:t * M + mb_ + nb, :])
                            for j in range(nb):
                                m = mb_ + j
                                if "aggmm" in ABLATE and m > 0:
                                    continue
                                nc.tensor.matmul(
                                    pm[0][:], lhsT=g[:, j * C1:(j + 1) * C1],
                                    rhs=oh[:, m * P:(m + 1) * P],
                                    start=(m == 0),
                                    stop=(m == M - 1 or "aggmm" in ABLATE))
                    elif "antgather" in OPT:
                        # dma_gather calls capped at KB2 chunks (<=768
                        # descriptors) so one call never exceeds the 1024-desc
                        # SWDGE carveout ring
                        Slo, Shi = MLO * 8, MHI * 8
                        chunk_src = []   # (g_tile, local_m) for m = 0..M-1
                        for view, idxsb, scol, Mx in (
                                (h_full[0:SPLIT, :], idxlo_sb, t * Slo, MLO),
                                (h_full[SPLIT:N, :], idxhi_sb, t * Shi, MHI)):
                            for base_ in range(0, Mx, KB2):
                                nb = min(KB2, Mx - base_)
                                g = gp.tile([P, KB2, HID], bf16, tag="gant",
                                            bufs=14 if "deep" in OPT else 8)
                                if "gather" in ABLATE:
                                    # same-volume sequential DMA in place of
                                    # the SWDGE random gather
                                    r0 = ((t * 131 + base_ * 17) % 300) * P
                                    nc.scalar.dma_start(
                                        g[:, :nb, :],
                                        h_full[r0:r0 + nb * P, :].rearrange(
                                            "(n p) d -> p n d", p=P))
                                else:
                                    _gq[0] = (_gq[0] + 1) % NSWQ
                                    nc.gpsimd.dma_gather(
                                        out_ap=g[:, :nb, :], in_ap=view,
                                        idxs_ap=idxsb[:, scol + base_ * 8:
                                                      scol + (base_ + nb) * 8],
                                        num_idxs=nb * P, num_idxs_reg=nb * P,
                                        elem_size=HID, queue_num=_gq[0])
                                chunk_src += [(g, j) for j in range(nb)]
                        for m in range(M):
                            if "aggmm" in ABLATE and m > 0:
                                continue
                            gsrc, mm = chunk_src[m]
                            for h in range(khalves):
                                nc.tensor.matmul(
                                    pm[h][:],
                                    lhsT=gsrc[:, mm, h * P:(h + 1) * P],
                                    rhs=oh[:, m * P:(m + 1) * P],
                                    start=(m == 0),
                                    stop=(m == M - 1 or "aggmm" in ABLATE))
                    elif "batchgather" in OPT:
                        for mb_ in range(0, M, KB2):
                            nb = min(KB2, M - mb_)
                            k = t * M + mb_
                            g = gp.tile([P, KB2 * HID], bf16, tag="gb", bufs=6)
                            nc.gpsimd.indirect_dma_start(
                                out=g[:, :nb * HID], out_offset=None,
                                in_=h_full[:],
                                in_offset=bass.IndirectOffsetOnAxis(
                                    ap=idx_sb[:, k:k + nb], axis=0))
                            for j in range(nb):
                                m = mb_ + j
                                if "aggmm" in ABLATE and m > 0:
                                    continue
                                for h in range(khalves):
                                    nc.tensor.matmul(
                                        pm[h][:],
                                        lhsT=g[:, j * HID + h * P:j * HID + (h + 1) * P],
                                        rhs=oh[:, m * P:(m + 1) * P],
                                        start=(m == 0),
                                        stop=(m == M - 1 or "aggmm" in ABLATE))
                    else:
                        for m in range(M):
                            k = t * M + m
                            g = gp.tile([P, HID], bf16, tag="g2")
                            if "gather" in ABLATE:
                                # same-volume sequential DMA instead of
                                # random indirect gather
                                nc.scalar.dma_start(
                                    g[:], h_full[(k % 390) * P:(k % 390) * P + P, :])
                            else:
                                nc.gpsimd.indirect_dma_start(
                                    out=g[:], out_offset=None, in_=h_full[:],
                                    in_offset=bass.IndirectOffsetOnAxis(
                                        ap=idx_sb[:, k:k + 1], axis=0))
                            if "aggmm" in ABLATE and m > 0:
                                continue
                            for h in range(khalves):
                                nc.tensor.matmul(
                                    pm[h][:], lhsT=g[:, h * P:(h + 1) * P],
                                    rhs=oh[:, m * P:(m + 1) * P],
                                    start=(m == 0),
                                    stop=(m == M - 1 or "aggmm" in ABLATE))
                    for h in range(khalves):
                        nc.vector.tensor_copy(
                            meanT_sb[h][:, t * P:(t + 1) * P], pm[h][:])

            def dense(layer, cin, src_feat, dst_feat):
                """dst_feat[j] = relu(Wl.T @ meanT + Wr.T @ src_feat + bl)."""
                khalves = cin // P
                for j in range(KH):
                    for b0, blen in nblocks:
                        pd = pdense.tile([P, 512], f32, tag="pd", space="PSUM")
                        nmm = 2 * khalves if "dense" not in ABLATE else 1
                        i = 0
                        for h in range(khalves):
                            nc.tensor.matmul(
                                pd[:, :blen],
                                lhsT=wl_sb[layer][h][:, j * P:(j + 1) * P],
                                rhs=meanT_sb[h][:, b0:b0 + blen],
                                start=(i == 0), stop=(i == nmm - 1)); i += 1
                            if "dense" in ABLATE:
                                break
                            nc.tensor.matmul(
                                pd[:, :blen],
                                lhsT=wr_sb[layer][h][:, j * P:(j + 1) * P],
                                rhs=src_feat[h][:, b0:b0 + blen],
                                start=(i == 0), stop=(i == nmm - 1)); i += 1
                        nc.vector.tensor_scalar(
                            out=dst_feat[j][:, b0:b0 + blen], in0=pd[:, :blen],
                            scalar1=bl_sb[:, 2 * (layer - 1) + j:2 * (layer - 1) + j + 1],
                            scalar2=0.0,
                            op0=mybir.AluOpType.add, op1=mybir.AluOpType.max)

            def write_rows(feat, h_full):
                """Transpose hT -> row-major h_own, then AllGather into h_full."""
                for t in range(TILES):
                    rows = stp.tile([P, HID], bf16, tag="rows")
                    for j in range(KH):
                        if "rows" in ABLATE:  # skip PE transpose, straight copy
                            nc.vector.tensor_copy(
                                rows[:, j * P:(j + 1) * P],
                                feat[j][:, t * P:(t + 1) * P])
                            continue
                        pt = ptr.tile([P, P], bf16, tag="pt", space="PSUM")
                        nc.tensor.transpose(
                            pt[:], feat[j][:, t * P:(t + 1) * P], identity[:])
                        nc.vector.tensor_copy(rows[:, j * P:(j + 1) * P], pt[:])
                    nrow = min(P, NP - t * P)
                    nc.scalar.dma_start(h_own[t * P:t * P + nrow, :], rows[:nrow, :])
                if h_full is not None and "coll" not in ABLATE:
                    nc.gpsimd.collective_compute(
                        "AllGather", mybir.AluOpType.bypass,
                        replica_groups=[list(range(NCORES))],
                        ins=[h_own[:]], outs=[h_full[:]])

            # ---- layer 1
            aggregate(1, C1)
            dense(1, C1, xT_sb, hT_sb)
            write_rows(hT_sb, h_full_for[2])
            # ---- layer 2
            aggregate(2, HID)
            dense(2, HID, hT_sb, xT_sb)   # ping-pong: xT_sb now holds h2T
            write_rows(xT_sb, h_full_for[3])
            # ---- layer 3
            aggregate(3, HID)
            dense(3, HID, xT_sb, hT_sb)   # hT_sb now holds h3T
            # ---- output layer: out[n, :] = h3.T @ Wo + bo
            for t in range(TILES):
                po = ptr.tile([P, OUT], f32, tag="po", space="PSUM")
                for h in range(KH):
                    nc.tensor.matmul(
                        po[:], lhsT=hT_sb[h][:, t * P:(t + 1) * P],
                        rhs=wo_sb[h][:],
                        start=(h == 0), stop=(h == KH - 1))
                orow = stp.tile([P, OUT], f32, tag="orow")
                nc.vector.tensor_tensor(out=orow[:], in0=po[:], in1=bo_sb[:],
                                        op=mybir.AluOpType.add)
                nrow = min(P, NP - t * P)
                nc.sync.dma_start(out_d[t * P:t * P + nrow, :], orow[:nrow, :])
    return nc


_PROGRAM_CACHE = {}


def _get_program(meta):
    key = (meta, frozenset(OPT), frozenset(ABLATE))
    if key not in _PROGRAM_CACHE:
        nc = bacc.Bacc("TRN2", target_bir_lowering=False, debug=False,
                       num_devices=NCORES, num_swdge_queues=NSWQ)
        _build(nc, *meta)
        nc.compile()
        _PROGRAM_CACHE[key] = nc
    return _PROGRAM_CACHE[key]


def make_in_maps(inputs):
    x = np.asarray(inputs["x"], np.float32)
    per_core, M, MLO, MHI = _preprocess(x, np.asarray(inputs["edge_index"]))
    iota = np.tile(np.arange(P, dtype=np.float32)[None, :], (P, 1)).astype(_bf)
    bl = np.zeros((P, 6), np.float32)
    for l in (1, 2, 3):
        b = np.asarray(inputs[f"bl{l}"], np.float32)
        bl[:, 2 * (l - 1)] = b[:P]
        bl[:, 2 * (l - 1) + 1] = b[P:]
    common = {"iota": iota, "bl": bl,
              "Wo": np.asarray(inputs["Wo"]).astype(_bf),
              "bo": np.asarray(inputs["bo"], np.float32).reshape(1, OUT)}
    for l in (1, 2, 3):
        common[f"Wl{l}"] = np.asarray(inputs[f"Wl{l}"]).astype(_bf)
        common[f"Wr{l}"] = np.asarray(inputs[f"Wr{l}"]).astype(_bf)
    in_maps = []
    for d in range(NCORES):
        pc = per_core[d]
        in_maps.append({**common, "idx32": pc["idx32"], "dstw": pc["dstw"],
                        "xg1": pc["xg1"], "xT": pc["xT"], "oh": pc["oh"],
                        "idx16lo": pc["idx16lo"], "idx16hi": pc["idx16hi"]})
    return in_maps, (M, MLO, MHI)


def kernel(**inputs) -> np.ndarray:
    in_maps, meta = make_in_maps(inputs)
    nc = _get_program(meta)
    res = run_bass_kernel_spmd(nc, in_maps, core_ids=list(range(NCORES)))
    out = np.concatenate(
        [np.asarray(res.results[d]["out"], np.float32) for d in range(NCORES)], axis=0)
    return out



# revision 50
# speedup vs baseline: 3.6565x; 1.5577x over previous
"""GraphSAGE-style 3-layer GNN (mean aggregation) on 8 Trainium2 NeuronCores.

Strategy (dst-sharded graph parallelism):
- Nodes (and their incoming edges) are sharded across 8 cores: core d owns
  dst nodes [d*6250, (d+1)*6250).
- Host preprocessing sorts each core's edges by dst node-tile (128 dst nodes
  per tile) and packs them into fixed 128-edge chunks on a uniform
  [49 tiles x M chunks] grid (padded with weight-0 edges).
- Aggregation on device: for each chunk, a one-hot matrix
  OH[e, n] = (dstloc[e] == n) * (1/deg) is built on the DVE from an iota tile,
  and  meanT[c, n] += G[e, c].T @ OH[e, n]  accumulates in PSUM on the PE.
- Layer-1 edge source features are pre-gathered on the host (edge-parallel
  input sharding) and streamed sequentially; layers 2/3 gather their source
  features from a replicated DRAM table via indirect DMA (128 rows/call).
- The replicated table is refreshed between layers with an AllGather
  (halo exchange) over the 8 cores.
- Dense transforms run in transposed layout: hT = Wl.T @ meanT + Wr.T @ xT,
  bias add + ReLU on the DVE, then PE-transposes produce row-major h for the
  next layer's gather table.
Compute dtype: bf16 inputs with fp32 PSUM accumulation.
"""
import numpy as np
import ml_dtypes

import concourse.bass as bass
import concourse.bacc as bacc
import concourse.mybir as mybir
import concourse.tile as tile
from concourse.bass_utils import run_bass_kernel_spmd

N = 50000
E = 800000
C1 = 128
HID = 256
OUT = 15
NCORES = 8
NP = N // NCORES          # 6250 own nodes per core
P = 128
TILES = (NP + P - 1) // P  # 49 node tiles per core
NPAD = TILES * P           # 6272
KB = 6                     # L1 staged-gather chunks per DMA

bf16 = mybir.dt.bfloat16
f32 = mybir.dt.float32
_bf = ml_dtypes.bfloat16
ABLATE = set()  # {"dense","rows","aggmm","gather","coll"} for timing experiments
# defaults: host-precomputed one-hot matrices streamed from DRAM instead of
# DVE builds, and int16 dma_gather (lo/hi split tables, <=768-descriptor calls)
# instead of per-chunk indirect DMAs for the layer-2/3 feature gathers
OPT = {"antgather", "deep", "sharedhf"}  # also: {"hostoh", "batchgather"}
NSWQ = 4        # SWDGE queues; dma_gather calls round-robin across them
KB2 = 6         # chunks per batched indirect gather (768 descriptors)
# The halo exchange is split into NGRP AllGathers over tile-aligned local-row
# groups; group g's collective is issued as soon as write_rows passes its
# last tile, so later groups' collectives overlap earlier groups' gathers.
NGRP = 2
GRP_TILES = [TILES // NGRP + (1 if i < TILES % NGRP else 0)
             for i in range(NGRP)]                      # [25, 24]
GRP_T0 = [sum(GRP_TILES[:i]) for i in range(NGRP)]      # first tile of group
GRP_R0 = [t0 * P for t0 in GRP_T0]                      # first local row
GRP_NR = [min((GRP_T0[g] + GRP_TILES[g]) * P, NP) - GRP_R0[g]
          for g in range(NGRP)]                          # rows per group
GRP_TBL = [NCORES * nr for nr in GRP_NR]                 # gather-table rows


def _pack16(a):
    """Pack [TILES, L] int row arrays into the dma_gather int16 index layout:
    position i lives at idx16[i % 16, i // 16], replicated to 128 partitions."""
    TILESn, L = a.shape
    S = L // 16
    out = np.zeros((TILESn, 16, S), np.int16)
    i = np.arange(L)
    out[:, i % 16, i // 16] = a.astype(np.int16)
    rep = np.tile(out, (1, 8, 1))                      # [TILES, 128, S]
    return np.ascontiguousarray(rep.transpose(1, 0, 2).reshape(128, TILESn * S))


def _preprocess(x, edge_index):
    """Sort/pad edges into the uniform [NCORES, TILES, M, 128] chunk grid and
    build all per-core staged arrays. Within each tile, chunks are grouped by
    the source's local-row GROUP (NGRP tile-aligned ranges), so layers 2/3
    gather each group from its own AllGathered table — letting later groups'
    collectives overlap earlier groups' gathers."""
    src = np.ascontiguousarray(edge_index[0]).astype(np.int64)
    dst = np.ascontiguousarray(edge_index[1]).astype(np.int64)
    deg = np.bincount(dst, minlength=N)
    wnode = (1.0 / np.maximum(deg, 1)).astype(np.float32)

    core = dst // NP
    tilei = (dst - core * NP) // P
    gkey = core * TILES + tilei
    lsrc = src % NP
    csrc = src // NP
    grp = np.searchsorted(np.asarray(GRP_R0), lsrc, side="right") - 1
    okey = gkey * NGRP + grp
    order = np.argsort(okey, kind="stable")
    so = okey[order]
    ssrc = src[order]
    sdst = dst[order]
    # per-group-table row ids for the sorted edges
    slsrc = lsrc[order]
    scsrc = csrc[order]
    sgrp = grp[order]
    r0 = np.asarray(GRP_R0)[sgrp]
    nr = np.asarray(GRP_NR)[sgrp]
    srowab = scsrc * nr + (slsrc - r0)
    bounds = np.searchsorted(so, np.arange(NCORES * TILES * NGRP))
    countsg = np.diff(np.append(bounds, E))
    M_list = [int((countsg[g::NGRP].max() + P - 1) // P) for g in range(NGRP)]
    MOFF = [sum(M_list[:g]) for g in range(NGRP)]
    M = sum(M_list)
    NCH = TILES * M

    # slot grids: group g edges at chunks [MOFF[g], MOFF[g]+M_list[g])
    rank = np.arange(E) - bounds[so]
    gk = so // NGRP
    slot = gk * (M * P) + np.asarray(MOFF)[sgrp] * P + rank
    src_grid = np.zeros(NCORES * NCH * P, np.int64)     # global rows (xg1)
    row_grid = np.zeros(NCORES * NCH * P, np.int64)     # A/B-table rows
    dstloc_grid = np.full(NCORES * NCH * P, P - 1, np.int64)
    # padding slots get dloc=255 (outside 0..127) so a binary is_equal one-hot
    # zeroes them without needing a per-edge weight
    dloc_pad_grid = np.full(NCORES * NCH * P, 255, np.int64)
    w_grid = np.zeros(NCORES * NCH * P, np.float32)
    src_grid[slot] = ssrc
    row_grid[slot] = srowab
    dstloc_grid[slot] = sdst - (sdst // NP) * NP - ((sdst - (sdst // NP) * NP) // P) * P
    dloc_pad_grid[slot] = dstloc_grid[slot]
    w_grid[slot] = wnode[sdst]

    src_grid = src_grid.reshape(NCORES, NCH, P)
    row_grid = row_grid.reshape(NCORES, NCH, P)
    dstloc_grid = dstloc_grid.reshape(NCORES, NCH, P)
    dloc_pad_grid = dloc_pad_grid.reshape(NCORES, NCH, P)
    w_grid = w_grid.reshape(NCORES, NCH, P)

    x_bf = x.astype(_bf)
    per_core = []
    for d in range(NCORES):
        # L1 pre-gathered edge features [128, NCH, C1]
        xg1 = np.ascontiguousarray(x_bf[src_grid[d]].transpose(1, 0, 2))
        # own transposed features [128, NPAD]
        xT = np.zeros((C1, NPAD), np.float32)
        xT[:, :NP] = x[d * NP:(d + 1) * NP].T
        if "hostoh" in OPT:
            # host-built one-hot scatter grid: oh[lane, chunk*P + dst]
            ohg = np.zeros((NCH, P, P), np.float32)
            ch = np.repeat(np.arange(NCH), P)
            ln = np.tile(np.arange(P), NCH)
            ohg[ch, ln, dstloc_grid[d].ravel()] = w_grid[d].ravel()
            ohg = np.ascontiguousarray(
                ohg.transpose(1, 0, 2).reshape(P, NCH * P)).astype(_bf)
        else:
            ohg = None
        # int16 dma_gather index planes (per-group-table row ids)
        s3 = row_grid[d].reshape(TILES, M, P)
        idxg = {f"idx16g{g}": _pack16(
                    s3[:, MOFF[g]:MOFF[g] + M_list[g], :].reshape(
                        TILES, M_list[g] * P))
                for g in range(NGRP)}
        # binary one-hot build inputs: per-slot dloc (pad=255) + per-node 1/deg
        dloc = np.ascontiguousarray(dloc_pad_grid[d].T).astype(_bf)  # [128, NCH]
        deginv = np.ones((1, NPAD), np.float32)
        deginv[0, :NP] = wnode[d * NP:(d + 1) * NP]
        per_core.append(dict(xg1=xg1, oh=ohg, idxg=idxg,
                             dloc=dloc, deginv=deginv.astype(_bf),
                             xT=np.ascontiguousarray(xT).astype(_bf)))
    return per_core, M, tuple(M_list)


def _build(nc: bass.Bass, M: int, M_list: tuple = ()):
    NCH = TILES * M
    MOFF = [sum(M_list[:g]) for g in range(NGRP)]
    # ---- I/O ----
    idxg_d = [nc.dram_tensor(f"idx16g{g}", [P, TILES * M_list[g] * 8],
                             mybir.dt.int16, kind="ExternalInput")
              for g in range(NGRP)]
    if "hostoh" in OPT:
        oh_d = nc.dram_tensor("oh", [P, NCH * P], bf16, kind="ExternalInput")
    else:
        dloc_d = nc.dram_tensor("dloc", [P, NCH], bf16, kind="ExternalInput")
        deginv_d = nc.dram_tensor("deginv", [1, NPAD], bf16,
                                  kind="ExternalInput")
        iota_d = nc.dram_tensor("iota", [P, P], bf16, kind="ExternalInput")
    xg1_d = nc.dram_tensor("xg1", [P, NCH, C1], bf16, kind="ExternalInput")
    xT_d = nc.dram_tensor("xT", [P, NPAD], bf16, kind="ExternalInput")
    wl_d = {}
    wr_d = {}
    for l, cin in ((1, C1), (2, HID), (3, HID)):
        wl_d[l] = nc.dram_tensor(f"Wl{l}", [cin, HID], bf16, kind="ExternalInput")
        wr_d[l] = nc.dram_tensor(f"Wr{l}", [cin, HID], bf16, kind="ExternalInput")
    bl_d = nc.dram_tensor("bl", [P, 6], f32, kind="ExternalInput")       # [:, 2(l-1)+j]
    wo_d = nc.dram_tensor("Wo", [HID, OUT], bf16, kind="ExternalInput")
    bo_d = nc.dram_tensor("bo", [1, OUT], f32, kind="ExternalInput")
    out_d = nc.dram_tensor("out", [NP, OUT], f32, kind="ExternalOutput")

    KH = HID // P  # 2 halves of the hidden dim
    nblocks = [(b, min(512, NPAD - b)) for b in range(0, NPAD, 512)]

    with tile.TileContext(nc) as tc:
        with (
            tc.tile_pool(name="const", bufs=1) as cst,
            tc.tile_pool(name="feat", bufs=1) as featp,
            tc.tile_pool(name="g", bufs=24) as gp,
            tc.tile_pool(name="oh", bufs=6 if "deep" in OPT else 4) as ohp,
            tc.tile_pool(name="stage", bufs=4) as stp,
            tc.tile_pool(name="pmean", bufs=2, space="PSUM") as pmean,
            tc.tile_pool(name="pdense", bufs=2, space="PSUM") as pdense,
            tc.tile_pool(name="ptr", bufs=1, space="PSUM") as ptr,
            tc.tile_pool(name="dram", bufs=1, space="DRAM") as dr,
        ):
            # ---- constants (funneled through DVE so consumers carry few waits)
            _cid = [0]
            def load_const(shape, dt, src_ap):
                _cid[0] += 1
                ld = cst.tile(shape, dt, tag=f"cld{_cid[0]}", name=f"cld{_cid[0]}")
                nc.sync.dma_start(ld[:], src_ap)
                t = cst.tile(shape, dt, tag=f"cst{_cid[0]}", name=f"cst{_cid[0]}")
                nc.vector.tensor_copy(t[:], ld[:])
                return t

            if "hostoh" not in OPT:
                iota_sb = load_const([P, P], bf16, iota_d[:])
                dloc_sb = cst.tile([P, NCH], bf16)
                nc.sync.dma_start(dloc_sb[:], dloc_d[:])
                # deginv broadcast to all 128 partitions via DMA step-0
                deginv_sb = cst.tile([P, NPAD], bf16)
                nc.sync.dma_start(deginv_sb[:], deginv_d[0:1, :].to_broadcast([P, NPAD]))
            idxg_sb = []
            for g in range(NGRP):
                t_ = cst.tile([P, TILES * M_list[g] * 8], mybir.dt.int16,
                              name=f"idxg{g}_sb")
                nc.sync.dma_start(t_[:], idxg_d[g][:])
                idxg_sb.append(t_)
            bl_sb = load_const([P, 6], f32, bl_d[:])
            wo_sb = [load_const([P, OUT], bf16, wo_d[h * P:(h + 1) * P, :])
                     for h in range(HID // P)]
            # bo broadcast to 128 partitions via DMA step-0
            bo_ld = cst.tile([P, OUT], f32)
            nc.sync.dma_start(bo_ld[:], bo_d[0:1, :].to_broadcast([P, OUT]))
            bo_sb = cst.tile([P, OUT], f32)
            nc.vector.tensor_copy(bo_sb[:], bo_ld[:])
            wl_sb = {}
            wr_sb = {}
            for l, cin in ((1, C1), (2, HID), (3, HID)):
                wl_sb[l] = [load_const([P, HID], bf16, wl_d[l][h * P:(h + 1) * P, :])
                            for h in range(cin // P)]
                wr_sb[l] = [load_const([P, HID], bf16, wr_d[l][h * P:(h + 1) * P, :])
                            for h in range(cin // P)]
            identity = cst.tile([P, P], bf16)
            from concourse.masks import make_identity
            make_identity(nc, identity[:])

            # ---- feature double buffers (transposed layout, [128, NPAD] per half)
            xT_sb = [featp.tile([P, NPAD], bf16, tag=f"ft0_{h}", name=f"xT_sb{h}") for h in range(KH)]
            hT_sb = [featp.tile([P, NPAD], bf16, tag=f"ft1_{h}", name=f"hT_sb{h}") for h in range(KH)]
            meanT_sb = [featp.tile([P, NPAD], bf16, tag=f"mt_{h}", name=f"meanT_sb{h}") for h in range(KH)]
            nc.sync.dma_start(xT_sb[0][:], xT_d[:])

            # DRAM halo tables, one per local-row group, so group g+1's
            # collective overlaps group g's gathers.
            # (ping-pong across layers: Shared tensors allow 1 writer each)
            hdt = mybir.dt.float8e4 if "fp8h" in OPT else bf16
            h_own = dr.tile([NP, HID], hdt)
            if "sharedhf" in OPT:
                h_ab_for = {
                    l: [dr.tile([GRP_TBL[g], HID], hdt, addr_space="Shared",
                                name=f"h_full{l}g{g}") for g in range(NGRP)]
                    for l in (2, 3)}
            else:
                htabs = [dr.tile([GRP_TBL[g], HID], hdt, name=f"h_fullg{g}")
                         for g in range(NGRP)]
                h_ab_for = {2: htabs, 3: htabs}

            _gq = [0]  # SWDGE queue round-robin state

            def build_oh(t, mb, mx):
                """One-hot for chunks [mb, mb+mx) of node-tile t: [128, mx*128]."""
                oh = ohp.tile([P, M * P], bf16, tag="oh")
                if "hostoh" in OPT:
                    # host-precomputed scatter matrices streamed from DRAM
                    nc.sync.dma_start(
                        oh[:, :mx * P],
                        oh_d[:, (t * M + mb) * P:(t * M + mb + mx) * P])
                    return oh
                # binary one-hot: padding slots have dloc=255 so they miss all
                # iota columns; the 1/deg weight is applied on the PSUM copy
                dloc = dloc_sb[:, t * M + mb:t * M + mb + mx]           # [128, mx]
                nc.vector.tensor_tensor(
                    out=oh[:, :mx * P].rearrange("p (m n) -> p m n", m=mx),
                    in0=dloc[:, :, None].to_broadcast([P, mx, P]),
                    in1=iota_sb[:, None, :].to_broadcast([P, mx, P]),
                    op=mybir.AluOpType.is_equal)
                return oh

            def store_mean(phase, last, t, pm, khalves):
                """PSUM partial sums -> meanT (pass 0 copy, pass 1 in-place
                add; 1/deg applied on the final pass unless hostoh)."""
                for h in range(khalves):
                    dstv = meanT_sb[h][:, t * P:(t + 1) * P]
                    if phase == 0:
                        nc.vector.tensor_copy(dstv, pm[h][:])
                    else:
                        nc.vector.tensor_tensor(
                            out=dstv, in0=dstv, in1=pm[h][:],
                            op=mybir.AluOpType.add)
                    if last and "hostoh" not in OPT:
                        nc.vector.tensor_tensor(
                            out=dstv, in0=dstv,
                            in1=deginv_sb[:, t * P:(t + 1) * P],
                            op=mybir.AluOpType.mult)

            def aggregate(layer, cin):
                """meanT_sb <- segment-mean of gathered source features.

                Two passes over tiles (group A then group B) so no B-group
                gather instruction — which must wait for collective B — is ever
                queued ahead of an A-group gather (Pool/PE head-of-line)."""
                khalves = cin // P
                h_ab = h_ab_for.get(layer)
                if layer == 1:
                    for t in range(TILES):
                        oh = build_oh(t, 0, M)
                        pm = [pmean.tile([P, P], f32, tag=f"pm{h}", space="PSUM",
                                         name=f"pm_{t}_{h}")
                              for h in range(khalves)]
                        for mb_ in range(0, M, KB):
                            nb = min(KB, M - mb_)
                            g = gp.tile([P, KB * C1], bf16, tag="g1", bufs=12)
                            nc.scalar.dma_start(
                                g[:, :nb * C1],
                                xg1_d[:, t * M + mb_:t * M + mb_ + nb, :])
                            for j in range(nb):
                                m = mb_ + j
                                if "aggmm" in ABLATE and m > 0:
                                    continue
                                nc.tensor.matmul(
                                    pm[0][:], lhsT=g[:, j * C1:(j + 1) * C1],
                                    rhs=oh[:, m * P:(m + 1) * P],
                                    start=(m == 0),
                                    stop=(m == M - 1 or "aggmm" in ABLATE))
                        store_mean(0, True, t, pm, khalves)
                    return
                assert "antgather" in OPT, (
                    "indirect_dma_start is miscompiled under SPMD "
                    "num_devices>1; only antgather is supported")
                groups = [(h_ab[g][:], idxg_sb[g], M_list[g] * 8, M_list[g])
                          for g in range(NGRP)]
                for phase, (view, idxsb, S_, Mx) in enumerate(groups):
                    for t in range(TILES):
                        oh = build_oh(t, MOFF[phase], Mx)
                        pm = [pmean.tile([P, P], f32, tag=f"pm{h}",
                                         space="PSUM",
                                         name=f"pm_{phase}_{t}_{h}")
                              for h in range(khalves)]
                        chunk_src = []
                        for base_ in range(0, Mx, KB2):
                            nb = min(KB2, Mx - base_)
                            g = gp.tile([P, KB2, HID], hdt, tag="gant",
                                        bufs=(14 if "hostoh" in OPT else 10)
                                        if "deep" in OPT else 8)
                            if "gather" in ABLATE:
                                # same-volume sequential DMA in place of
                                # the SWDGE random gather
                                r0 = ((t * 131 + base_ * 17) % 150) * P
                                nc.scalar.dma_start(
                                    g[:, :nb, :],
                                    view[r0:r0 + nb * P, :].rearrange(
                                        "(n p) d -> p n d", p=P))
                            else:
                                _gq[0] = (_gq[0] + 1) % NSWQ
                                nc.gpsimd.dma_gather(
                                    out_ap=g[:, :nb, :], in_ap=view,
                                    idxs_ap=idxsb[:, t * S_ + base_ * 8:
                                                  t * S_ + (base_ + nb) * 8],
                                    num_idxs=nb * P, num_idxs_reg=nb * P,
                                    elem_size=HID, queue_num=_gq[0])
                            chunk_src += [(g, j) for j in range(nb)]
                        for m in range(Mx):
                            if "aggmm" in ABLATE and m > 0:
                                continue
                            gsrc, mm = chunk_src[m]
                            for h in range(khalves):
                                nc.tensor.matmul(
                                    pm[h][:],
                                    lhsT=gsrc[:, mm, h * P:(h + 1) * P],
                                    rhs=oh[:, m * P:(m + 1) * P],
                                    start=(m == 0),
                                    stop=(m == Mx - 1 or "aggmm" in ABLATE))
                        store_mean(phase, phase == NGRP - 1, t, pm, khalves)

            def dense(layer, cin, src_feat, dst_feat):
                """dst_feat[j] = relu(Wl.T @ meanT + Wr.T @ src_feat + bl)."""
                khalves = cin // P
                for j in range(KH):
                    for b0, blen in nblocks:
                        pd = pdense.tile([P, 512], f32, tag="pd", space="PSUM")
                        nmm = 2 * khalves if "dense" not in ABLATE else 1
                        i = 0
                        for h in range(khalves):
                            nc.tensor.matmul(
                                pd[:, :blen],
                                lhsT=wl_sb[layer][h][:, j * P:(j + 1) * P],
                                rhs=meanT_sb[h][:, b0:b0 + blen],
                                start=(i == 0), stop=(i == nmm - 1)); i += 1
                            if "dense" in ABLATE:
                                break
                            nc.tensor.matmul(
                                pd[:, :blen],
                                lhsT=wr_sb[layer][h][:, j * P:(j + 1) * P],
                                rhs=src_feat[h][:, b0:b0 + blen],
                                start=(i == 0), stop=(i == nmm - 1)); i += 1
                        nc.vector.tensor_scalar(
                            out=dst_feat[j][:, b0:b0 + blen], in0=pd[:, :blen],
                            scalar1=bl_sb[:, 2 * (layer - 1) + j:2 * (layer - 1) + j + 1],
                            scalar2=0.0,
                            op0=mybir.AluOpType.add, op1=mybir.AluOpType.max)

            def write_rows(feat, h_ab):
                """Transpose hT -> row-major h_own; AllGather each local-row
                group as soon as its last tile is written."""
                grp_end = {GRP_T0[g] + GRP_TILES[g] - 1: g for g in range(NGRP)}
                for t in range(TILES):
                    rows = stp.tile([P, HID], hdt, tag="rows")
                    for j in range(KH):
                        if "rows" in ABLATE:  # skip PE transpose, straight copy
                            nc.vector.tensor_copy(
                                rows[:, j * P:(j + 1) * P],
                                feat[j][:, t * P:(t + 1) * P])
                            continue
                        pt = ptr.tile([P, P], bf16, tag="pt", space="PSUM")
                        nc.tensor.transpose(
                            pt[:], feat[j][:, t * P:(t + 1) * P], identity[:])
                        nc.vector.tensor_copy(rows[:, j * P:(j + 1) * P], pt[:])
                    nrow = min(P, NP - t * P)
                    nc.scalar.dma_start(h_own[t * P:t * P + nrow, :], rows[:nrow, :])
                    g = grp_end.get(t)
                    if g is not None and h_ab is not None and "coll" not in ABLATE:
                        nc.gpsimd.collective_compute(
                            "AllGather", mybir.AluOpType.bypass,
                            replica_groups=[list(range(NCORES))],
                            ins=[h_own[GRP_R0[g]:GRP_R0[g] + GRP_NR[g], :]],
                            outs=[h_ab[g][:]])

            # ---- layer 1
            aggregate(1, C1)
            dense(1, C1, xT_sb, hT_sb)
            write_rows(hT_sb, h_ab_for[2])
            # ---- layer 2
            aggregate(2, HID)
            dense(2, HID, hT_sb, xT_sb)   # ping-pong: xT_sb now holds h2T
            write_rows(xT_sb, h_ab_for[3])
            # ---- layer 3
            aggregate(3, HID)
            dense(3, HID, xT_sb, hT_sb)   # hT_sb now holds h3T
            # ---- output layer: out[n, :] = h3.T @ Wo + bo
            for t in range(TILES):
                po = ptr.tile([P, OUT], f32, tag="po", space="PSUM")
                for h in range(KH):
                    nc.tensor.matmul(
                        po[:], lhsT=hT_sb[h][:, t * P:(t + 1) * P],
                        rhs=wo_sb[h][:],
                        start=(h == 0), stop=(h == KH - 1))
                orow = stp.tile([P, OUT], f32, tag="orow")
                nc.vector.tensor_tensor(out=orow[:], in0=po[:], in1=bo_sb[:],
                                        op=mybir.AluOpType.add)
                nrow = min(P, NP - t * P)
                nc.sync.dma_start(out_d[t * P:t * P + nrow, :], orow[:nrow, :])
    return nc


_PROGRAM_CACHE = {}


def _get_program(meta):
    key = (meta, frozenset(OPT), frozenset(ABLATE), KB2, NSWQ, NGRP)
    if key not in _PROGRAM_CACHE:
        nc = bacc.Bacc("TRN2", target_bir_lowering=False, debug=False,
                       num_devices=NCORES, num_swdge_queues=NSWQ)
        _build(nc, *meta)
        nc.compile()
        _PROGRAM_CACHE[key] = nc
    return _PROGRAM_CACHE[key]


def make_in_maps(inputs):
    x = np.asarray(inputs["x"], np.float32)
    per_core, M, M_list = _preprocess(x, np.asarray(inputs["edge_index"]))
    iota = np.tile(np.arange(P, dtype=np.float32)[None, :], (P, 1)).astype(_bf)
    bl = np.zeros((P, 6), np.float32)
    for l in (1, 2, 3):
        b = np.asarray(inputs[f"bl{l}"], np.float32)
        bl[:, 2 * (l - 1)] = b[:P]
        bl[:, 2 * (l - 1) + 1] = b[P:]
    common = {"bl": bl,
              "Wo": np.asarray(inputs["Wo"]).astype(_bf),
              "bo": np.asarray(inputs["bo"], np.float32).reshape(1, OUT)}
    for l in (1, 2, 3):
        common[f"Wl{l}"] = np.asarray(inputs[f"Wl{l}"]).astype(_bf)
        common[f"Wr{l}"] = np.asarray(inputs[f"Wr{l}"]).astype(_bf)
    in_maps = []
    for d in range(NCORES):
        pc = per_core[d]
        im = {**common, "xg1": pc["xg1"], "xT": pc["xT"], **pc["idxg"]}
        if "hostoh" in OPT:
            im["oh"] = pc["oh"]
        else:
            im.update(dloc=pc["dloc"], deginv=pc["deginv"], iota=iota)
        in_maps.append(im)
    return in_maps, (M, M_list)


def kernel(**inputs) -> np.ndarray:
    in_maps, meta = make_in_maps(inputs)
    nc = _get_program(meta)
    res = run_bass_kernel_spmd(nc, in_maps, core_ids=list(range(NCORES)))
    out = np.concatenate(
        [np.asarray(res.results[d]["out"], np.float32) for d in range(NCORES)], axis=0)
    return out

